# revision 13
# baseline (speedup 1.0000x reference)
"""Bahdanau attention on TRN2 — data-parallel over batch across 8 NeuronCores.

Math per batch row n (shapes: T=2048 encoder steps, E=U=1024):
    K_projT[u, t] = sum_e Wk[u, e] * X[n, t, e]          (big matmul, [U, T] layout)
    th[u, t]      = tanh(K_projT[u, t] + q_proj[n, u])   (ACT, per-partition bias)
    scores[t]     = sum_u v[u] * th[u, t]                (PE, v as 1-col stationary)
    a[t]          = softmax(scores + mask[n])            (mask additive -1e30)
    ctx[e]        = sum_t a[t] * X[n, t, e]              (PE, aT cols as stationary)

Host precomputes q_proj = queries @ Wq.T (tiny), the additive mask from
`lengths`, X transposed per row ([E, T]) so the contraction dim lands on
SBUF partitions, plus small layout shuffles of Wk / v / q_proj.
"""

import numpy as np

import concourse.bass as bass
import concourse.mybir as mybir
import concourse.tile as tile
from concourse.bass_utils import run_bass_kernel_spmd

# Problem shape (hardcoded per contract; kernel.py must be self-contained).
N, T, D_ENC, D_DEC, U = 32, 2048, 1024, 1024, 1024
N_CORES = 8
R = N // N_CORES            # batch rows per core
P = 128                     # SBUF partitions
TC = 512                    # t-chunk = matmul moving free dim (fp32 max)
NTC = T // TC
ET = D_ENC // P             # e-tiles (contraction of the big matmul)
UT = U // P                 # u-tiles
TT = T // P                 # t-tiles (contraction of the context matmul)
EC = 512
NEC = D_ENC // EC

F32 = mybir.dt.float32
# PE matmul dtype. float32r = single-pass fp32 matmul (full rate at free
# dim >= 256); plain float32 = 2 half-speed passes (4x slower).
MM_DT = mybir.dt.float32r

AF = mybir.ActivationFunctionType
AX = mybir.AxisListType

MASK_NEG = np.float32(-1.0e30)

LAST_RESULTS = None         # BassKernelResults of the most recent run
_PROGRAM = None


def _mm(ap):
    return ap if ap.dtype == MM_DT else ap.bitcast(MM_DT)


def _legalize_waits(nc):
    """Several walrus instruction encodings (the self-loading fp32r matmul's
    S3_LW, Activation's S3D3_AC, ...) have a single sync-wait slot, but Tile
    sometimes emits 2+ waits on one instruction. Hoist the extra waits onto
    engine NoOps inserted just before the instruction — the engine's NX
    evaluates waits in program order, so gating is preserved. This covers
    HWDGE DMAs too: the issuing engine's sequencer writes the descriptor
    in program order, so a same-engine NoOp gates the transfer."""
    for f in nc.m.functions:
        for blk in f.blocks:
            insts = blk.instructions
            idx = 0
            while idx < len(insts):
                ins = insts[idx]
                if (
                    not isinstance(ins, mybir.InstCollectiveCompute)
                    and ins.engine is not None
                    and ins.sync_info is not None
                    and len(ins.sync_info.on_wait) > 1
                ):
                    waits = list(ins.sync_info.on_wait)
                    # one wait per NoOp — every ISA ctrl struct fits that
                    for w in waits[1:]:
                        nop = mybir.InstNoOp(
                            name=nc.get_next_instruction_name(), ins=[], outs=[]
                        )
                        nop.engine = ins.engine
                        nop.sync_info = mybir.SyncInfo(on_wait=[w], on_update=[])
                        insts.insert(idx, nop)
                        idx += 1
                    ins.sync_info = mybir.SyncInfo(
                        on_wait=[waits[0]], on_update=list(ins.sync_info.on_update)
                    )
                idx += 1


def build_program(legalize: bool = True) -> bass.Bass:
    nc = bass.Bass("TRN2")

    xt = nc.dram_tensor("xt", [R, D_ENC, T], F32, kind="ExternalInput").ap()
    xn = nc.dram_tensor("xn", [R, T, D_ENC], F32, kind="ExternalInput").ap()
    wkt = nc.dram_tensor("wkt", [D_ENC, U], F32, kind="ExternalInput").ap()
    vt = nc.dram_tensor("vt", [P, UT], F32, kind="ExternalInput").ap()
    qpt = nc.dram_tensor("qpt", [P, R * UT], F32, kind="ExternalInput").ap()
    mask = nc.dram_tensor("mask", [R, T], F32, kind="ExternalInput").ap()
    ctx_out = nc.dram_tensor("contexts", [R, D_ENC], F32, kind="ExternalOutput").ap()
    align_out = nc.dram_tensor("alignments", [R, T], F32, kind="ExternalOutput").ap()

    with tile.TileContext(nc) as tc:
        with (
            tc.tile_pool(name="const", bufs=1) as const_pool,
            tc.tile_pool(name="xtp", bufs=3) as xt_pool,
            tc.tile_pool(name="thp", bufs=4) as th_pool,
            tc.tile_pool(name="xnp", bufs=6) as xn_pool,
            tc.tile_pool(name="rowp", bufs=2) as row_pool,
            tc.tile_pool(name="psm", bufs=3, space="PSUM") as psum_m,
            tc.tile_pool(name="pss", bufs=2, space="PSUM") as psum_s,
            tc.tile_pool(name="psc", bufs=2, space="PSUM") as psum_c,
            tc.tile_pool(name="dramp", bufs=2, space="DRAM") as dram_pool,
        ):
            # Replicated constants, resident for the whole kernel.
            wk_sb = const_pool.tile([P, ET, U], MM_DT, tag="wk")
            for et in range(ET):
                nc.sync.dma_start(wk_sb[:, et], _mm(wkt[et * P:(et + 1) * P, :]))
            vt_sb = const_pool.tile([P, UT], MM_DT, tag="vt")
            nc.sync.dma_start(vt_sb[:], _mm(vt[:, :]))
            qpt_sb = const_pool.tile([P, R * UT], F32, tag="qpt")
            nc.sync.dma_start(qpt_sb[:], qpt[:, :])

            for r in range(R):
                mask_sb = row_pool.tile([1, T], F32, tag="mask")
                nc.sync.dma_start(mask_sb[:], mask[r:r + 1, :])
                scores_sb = row_pool.tile([1, T], F32, tag="scores")

                for c in range(NTC):
                    xt_sb = xt_pool.tile([P, ET, TC], MM_DT, tag="xt")
                    for et in range(ET):
                        nc.sync.dma_start(
                            xt_sb[:, et],
                            _mm(xt[r, et * P:(et + 1) * P, c * TC:(c + 1) * TC]),
                        )
                    sc_ps = psum_s.tile([1, TC], F32, tag="sc")
                    ths = []
                    for ut in range(UT):
                        ps = psum_m.tile([P, TC], F32, tag="kproj")
                        for et in range(ET):
                            nc.tensor.matmul(
                                ps[:],
                                _mm(wk_sb[:, et, ut * P:(ut + 1) * P]),
                                _mm(xt_sb[:, et]),
                                start=(et == 0),
                                stop=(et == ET - 1),
                            )
                        th = th_pool.tile([P, TC], MM_DT, tag="th")
                        nc.scalar.activation(
                            th[:], ps[:], AF.Tanh,
                            bias=qpt_sb[:, r * UT + ut:r * UT + ut + 1],
                        )
                        ths.append(th)
                        # score matmul for ut-1: one main group behind, so the
                        # tanh it waits on is already finished (no PE stall).
                        if ut > 0:
                            nc.tensor.matmul(
                                sc_ps[:], _mm(vt_sb[:, ut - 1:ut]), _mm(ths[ut - 1][:]),
                                start=(ut == 1), stop=False,
                            )
                    nc.tensor.matmul(
                        sc_ps[:], _mm(vt_sb[:, UT - 1:UT]), _mm(ths[UT - 1][:]),
                        start=False, stop=True,
                    )
                    nc.vector.tensor_add(
                        scores_sb[:, c * TC:(c + 1) * TC], sc_ps[:],
                        mask_sb[:, c * TC:(c + 1) * TC],
                    )

                # Softmax over the [1, T] score vector (partition 0).
                mxn = row_pool.tile([1, 1], F32, tag="mxn")
                nc.vector.reduce_max(mxn[:], scores_sb[:], axis=AX.X, negate=True)
                exp_sb = row_pool.tile([1, T], F32, tag="exp")
                zsum = row_pool.tile([1, 1], F32, tag="z")
                nc.scalar.activation(
                    exp_sb[:], scores_sb[:], AF.Exp, bias=mxn[:], accum_out=zsum[:],
                )
                rz = row_pool.tile([1, 1], F32, tag="rz")
                nc.vector.reciprocal(rz[:], zsum[:])
                align_sb = row_pool.tile([1, T], F32, tag="align")
                nc.scalar.activation(align_sb[:], exp_sb[:], AF.Copy, scale=rz[:])
                nc.sync.dma_start(align_out[r:r + 1, :], align_sb[:])

                # Transpose the (unnormalized) attention vector to the
                # partition dim via a DRAM bounce; normalization is folded
                # into the final context scale instead.
                bounce = dram_pool.tile([1, T], F32, tag="bounce")
                nc.sync.dma_start(bounce[:], exp_sb[:])
                at_sb = row_pool.tile([P, TT], MM_DT, tag="at")
                nc.sync.dma_start(at_sb[:], _mm(bounce[0].rearrange("(j p) -> p j", p=P)))

                # ctx[e] = (1/Z) * sum_t exp[t] * X[t, e]
                ct_ps0 = psum_c.tile([1, EC], F32, tag="ctx")
                ct_ps1 = psum_c.tile([1, EC], F32, tag="ctx")
                for tt in range(TT):
                    xn_sb = xn_pool.tile([P, D_ENC], MM_DT, tag="xn")
                    nc.sync.dma_start(xn_sb[:], _mm(xn[r, tt * P:(tt + 1) * P, :]))
                    nc.tensor.matmul(
                        ct_ps0[:], _mm(at_sb[:, tt:tt + 1]), _mm(xn_sb[:, 0:EC]),
                        start=(tt == 0), stop=(tt == TT - 1),
                    )
                    nc.tensor.matmul(
                        ct_ps1[:], _mm(at_sb[:, tt:tt + 1]), _mm(xn_sb[:, EC:2 * EC]),
                        start=(tt == 0), stop=(tt == TT - 1),
                    )
                ctx_sb = row_pool.tile([1, D_ENC], F32, tag="ctx_sb")
                nc.scalar.activation(ctx_sb[:, 0:EC], ct_ps0[:], AF.Copy, scale=rz[:])
                nc.scalar.activation(ctx_sb[:, EC:], ct_ps1[:], AF.Copy, scale=rz[:])
                nc.sync.dma_start(ctx_out[r:r + 1, :], ctx_sb[:])

    if legalize:
        _legalize_waits(nc)
    return nc


def _get_program() -> bass.Bass:
    global _PROGRAM
    if _PROGRAM is None:
        _PROGRAM = build_program()
    return _PROGRAM


def make_in_maps(queries, encoder_output, lengths, v, Wq, Wk):
    """Host-side marshalling: shard batch across cores + layout shuffles."""
    queries = np.ascontiguousarray(np.asarray(queries), dtype=np.float32)
    encoder_output = np.ascontiguousarray(np.asarray(encoder_output), dtype=np.float32)
    lengths = np.asarray(lengths).astype(np.int64)
    v = np.asarray(v, dtype=np.float32)
    Wq = np.asarray(Wq, dtype=np.float32)
    Wk = np.asarray(Wk, dtype=np.float32)

    qp = queries[:, 0, :] @ Wq.T                                   # [N, U]
    xt_full = np.ascontiguousarray(encoder_output.transpose(0, 2, 1))  # [N, E, T]
    wkt = np.ascontiguousarray(Wk.T)                               # [E, U]
    vt = np.ascontiguousarray(v.reshape(UT, P).T)                  # [P, UT]
    mask = np.where(
        np.arange(T)[None, :] >= lengths[:, None], MASK_NEG, np.float32(0.0)
    ).astype(np.float32)                                           # [N, T]

    in_maps = []
    for i in range(N_CORES):
        sl = slice(i * R, (i + 1) * R)
        qpt = np.ascontiguousarray(
            qp[sl].reshape(R, UT, P).transpose(2, 0, 1).reshape(P, R * UT)
        )
        in_maps.append({
            "xt": xt_full[sl],
            "xn": encoder_output[sl],
            "wkt": wkt,
            "vt": vt,
            "qpt": qpt,
            "mask": np.ascontiguousarray(mask[sl]),
        })
    return in_maps


def kernel(queries, encoder_output, lengths, v, Wq, Wk, _trace=False):
    global LAST_RESULTS
    in_maps = make_in_maps(queries, encoder_output, lengths, v, Wq, Wk)
    nc = _get_program()
    res = run_bass_kernel_spmd(
        nc, in_maps, core_ids=list(range(N_CORES)), trace=_trace
    )
    LAST_RESULTS = res
    contexts = np.concatenate(
        [res.results[i]["contexts"] for i in range(N_CORES)], axis=0
    )
    alignments = np.concatenate(
        [res.results[i]["alignments"] for i in range(N_CORES)], axis=0
    )
    return contexts, alignments
